# revision 36
# baseline (speedup 1.0000x reference)
"""MoE gate (router) kernel for Trainium2, 8 NeuronCores, data-parallel.

reference: logits = x @ W_g  ([16384,2048] @ [2048,64]); scores = softmax(logits);
           return top-6 (indices, scores).

Strategy (v2)
-------------
Data-parallel over tokens: each core handles 2048 tokens. The kernel is
HBM-bandwidth bound, so x is shipped at 3 bytes/element instead of 4:

    x  = xh (fp16)  +  2^-11 * xl'          xl' = (x - xh) * 2^11  in fp8 e4m3
    W  = Wh (fp16)  +  2^-11 * Wl'          Wl' = (W - Wh) * 2^11  in fp16
    logits = xh@Wh + 2^-11 * (xh@Wl' + xl'@e4m3(Wh))

All products accumulate exactly in fp32 PSUM; residual logit error ~2^-15
relative (measured: 4/16384 rows with a top-6 order flip, val err ~1e-5).

Per 128-token block (16 blocks/core):
  - ONE dma_start moves a [128 part, 6144 B] u8 slab (4 KiB fp16 hi +
    2 KiB fp8 lo per partition). ALL x slabs stream in order on the sync
    HWDGE ring -- a single ring sustains ~320 GB/s (HBM cap), and keeping
    dma_starts off the scalar engine means ACT work is never head-of-line
    blocked behind a stalled dma instruction. Blocks 12-13 split hi/lo
    into two DMAs and blocks 14-15 into quarters, so the tail matmuls
    chase the stream chunk-by-chunk (last MM retires ~0.7us after the
    final byte instead of ~1.2us). All slabs are SBUF-resident (no
    reuse), so DMA is never back-pressured by compute.
  - 16 matmuls  xh_c @ [Wh|Wl']_c (fp16, N=128) -> PSUM[:, 0:128]
  - 16 matmuls  xl'_c @ Wh8_c     (fp8,  N=64)  -> PSUM[:, 128:192]
  - fold on DVE with a single-AP pair reduce: u = sum over PSUM[:, 64:192]
    viewed as [P, 64, 2]; lg = u * 2^-11 + PSUM[:, :64]  (one PSUM operand
    per DVE op -- PSUM has a single DVE read port).
  - softmax+top6 without max-subtraction (|logits| < ~6): exp+rowsum fused
    on the scalar engine (accum_out) runs IN PARALLEL with DVE
    max8/find_index8 on the raw logits (indices go straight into the
    staging tile); scores = exp(v6) * recip(sum).
  - Outputs staged in SBUF [P, nt, 8]/[P, nt, 6]; two DMAs at the very
    end (mid-stream output DMAs measurably disturb the tile schedule).

Measured: 71.9 us (fp16 hi/lo baseline) -> 52.5-53.5 us (best 52460 ns).
Breakdown: ~8 us fixed framework preamble, ~38.5 us HBM-bound stream @
~327 GB/s, ~4 us compute tail + output DMA latency, ~2 us exit barrier
(the NRT semaphore-zeroing ladder after it is excluded from the reported
exec time).
"""

import os
import sys

import numpy as np

for _p in ("/opt/trn_rl_repo", "/root/.axon_site/_ro/trn_rl_repo"):
    if os.path.isdir(_p) and _p not in sys.path:
        sys.path.insert(0, _p)

import ml_dtypes
import concourse.bass as bass
import concourse.mybir as mybir
from concourse import bacc, bass_utils
from concourse.tile import TileContext

N_CORES = 8
T_FULL = 16384
K = 2048
E = 64
TOPK = 6
P = 128
KC = K // P  # 16 contraction chunks
LO_SCALE = 2048.0  # 2^11
FP8 = ml_dtypes.float8_e4m3

# per-block slab layout (bytes per partition)
HI_B = KC * P * 2  # 4096 B fp16 hi
LO_B = KC * P * 1  # 2048 B fp8 lo
BLK_B = HI_B + LO_B  # 6144
# W slab layout (bytes per partition); the fp8 copy of Wh is derived
# on-device (one DVE cast) instead of spending HBM stream time on it
W16_B = KC * 2 * E * 2  # 4096 B fp16 [Wh|Wl'] interleaved per chunk
W_B = W16_B

_NC_CACHE: dict[int, "bass.Bass"] = {}
LAST_RESULT = None  # BassKernelResults of the most recent kernel() call


def build_nc(t_shard: int = T_FULL // N_CORES) -> "bass.Bass":
    f16 = mybir.dt.float16
    f8 = mybir.dt.float8e4
    f32 = mybir.dt.float32
    u8 = mybir.dt.uint8
    u32 = mybir.dt.uint32
    EXP = mybir.ActivationFunctionType.Exp
    ADD = mybir.AluOpType.add
    MULT = mybir.AluOpType.mult

    assert t_shard % P == 0
    nt = t_shard // P  # number of 128-token blocks

    nc = bacc.Bacc()
    xd = nc.dram_tensor("xd", [P, nt * BLK_B], u8, kind="ExternalInput")
    Wd = nc.dram_tensor("Wd", [P, W_B], u8, kind="ExternalInput")
    idx_o = nc.dram_tensor("idx", [P, nt, 8], u32, kind="ExternalOutput")
    val_o = nc.dram_tensor("val", [P, nt, TOPK], f32, kind="ExternalOutput")

    with TileContext(nc) as tc:
        with (
            tc.tile_pool(name="singles", bufs=1) as singles,
            tc.tile_pool(name="psum", bufs=8, space="PSUM") as psum_pool,
        ):
            # one big resident x slab; 16 DMAs into disjoint slices (deps are
            # range-tracked, so readers only wait on their own block's DMA)
            xall = singles.tile([P, nt * BLK_B], u8)
            W_sb = singles.tile([P, W_B], u8)
            # W alone on the scalar ring (lands fast, scalar engine then only
            # runs ACTs); ALL x blocks stream in order on the sync ring so no
            # compute engine is ever head-of-line blocked by a dma_start.
            # 256 B head-DMAs warm each ring's DGE pipeline so the real
            # transfers behind them skip most of the cold-start latency
            nc.scalar.dma_start(out=W_sb[:, :256], in_=Wd[:, :256])
            nc.scalar.dma_start(out=W_sb[:, 256:], in_=Wd[:, 256:])
            nc.sync.dma_start(out=xall[:, :256], in_=xd[:, :256])
            for b in range(nt):
                o = b * BLK_B
                if b == 0:
                    nc.sync.dma_start(
                        out=xall[:, 256:BLK_B], in_=xd[:, 256:BLK_B]
                    )
                elif b < nt - 4:
                    nc.sync.dma_start(
                        out=xall[:, o : o + BLK_B], in_=xd[:, o : o + BLK_B]
                    )
                elif b < nt - 2:
                    # tail blocks: hi and lo parts land separately so the
                    # fp16 matmuls can start before the fp8 half arrives
                    nc.sync.dma_start(
                        out=xall[:, o : o + HI_B], in_=xd[:, o : o + HI_B]
                    )
                    nc.sync.dma_start(
                        out=xall[:, o + HI_B : o + BLK_B],
                        in_=xd[:, o + HI_B : o + BLK_B],
                    )
                else:
                    # last 2 blocks: hi in halves, lo in quarters, so the
                    # final matmuls chase the stream piece-by-piece
                    cuts = (
                        0, HI_B // 2, HI_B,
                        HI_B + LO_B // 4, HI_B + LO_B // 2,
                        HI_B + 3 * LO_B // 4, BLK_B,
                    )
                    for q0, q1 in zip(cuts[:-1], cuts[1:]):
                        nc.sync.dma_start(
                            out=xall[:, o + q0 : o + q1],
                            in_=xd[:, o + q0 : o + q1],
                        )
            # [P, KC, 128] fp16: per chunk cols 0:64 = Wh, 64:128 = Wl'
            W16 = W_sb[:, :W16_B].bitcast(f16).rearrange(
                "p (c n) -> p c n", c=KC
            )
            # [P, KC, 64] fp8 = e4m3(Wh), cast on-device (DVE RNE)
            W8_sb = singles.tile([P, KC, E], f8)
            nc.vector.tensor_copy(W8_sb, W16[:, :, :E])
            W8 = W8_sb[:]

            istage = singles.tile([P, nt, 8], u32)
            vstage = singles.tile([P, nt, TOPK], f32)
            # per-block scratch slices of one tile: u(64) lg(64) erow(64)
            # v8(8) ev(8) sume(1) rec(1) -> 212 f32, padded to 224
            SCR = 224
            scratch = singles.tile([P, nt, SCR], f32)

            for b in range(nt):
                xb = xall[:, b * BLK_B : (b + 1) * BLK_B]
                hi = xb[:, :HI_B].bitcast(f16).rearrange("p (c t) -> p c t", c=KC)
                lo = xb[:, HI_B:].bitcast(f8).rearrange("p (c t) -> p c t", c=KC)
                ps = psum_pool.tile([P, 192], f32, tag="ps")
                for c in range(KC):
                    nc.tensor.matmul(
                        ps[:, :128],
                        hi[:, c],
                        W16[:, c],
                        start=(c == 0),
                        stop=(c == KC - 1),
                    )
                for c in range(KC):
                    nc.tensor.matmul(
                        ps[:, 128:],
                        lo[:, c],
                        W8[:, c],
                        start=(c == 0),
                        stop=(c == KC - 1),
                    )
                sc = scratch[:, b]
                u = sc[:, 0:64]
                lg = sc[:, 64:128]
                erow = sc[:, 128:192]
                v8 = sc[:, 192:200]
                ev = sc[:, 200:208]
                sume = sc[:, 208:209]
                rec = sc[:, 209:210]
                # fold: u = ps[:,64:128] + ps[:,128:192] via one strided AP
                nc.vector.tensor_reduce(
                    u,
                    ps[:, 64:].rearrange("p (r j) -> p j r", r=2),
                    axis=mybir.AxisListType.X,
                    op=ADD,
                )
                nc.vector.scalar_tensor_tensor(
                    out=lg,
                    in0=u,
                    scalar=1.0 / LO_SCALE,
                    in1=ps[:, :E],
                    op0=MULT,
                    op1=ADD,
                )
                # softmax + top-6 (no max subtraction; |logits| < ~6).
                # max8/find_index8 run on lg (DVE) IN PARALLEL with the
                # exp+rowsum on the scalar engine; the top-6 VALUES come
                # from a second max8 on erow (exp is monotonic, so the
                # sorted values are exactly exp(v_i) from the same ACT
                # table -- bit-identical to exp'ing v6, one less ACT op
                # and one less cross-engine hop on the val critical path).
                nc.scalar.activation(erow, lg, EXP, accum_out=sume)
                nc.vector.max(out=v8, in_=lg)
                nc.vector.max_index(out=istage[:, b], in_max=v8, in_values=lg)
                nc.vector.max(out=ev, in_=erow)
                nc.vector.reciprocal(rec, sume)
                nc.vector.tensor_scalar_mul(vstage[:, b], ev[:, :TOPK], rec)

            # outputs at the end on the two HWDGE rings (both idle by now).
            # Split bulk (blocks 0..nt-2, deps already satisfied -> descriptor
            # generation overlaps the last block's chain) from a tiny final
            # DMA for block nt-1 that rides the already-warm DGE pipeline.
            nc.scalar.dma_start(out=val_o[:, : nt - 1], in_=vstage[:, : nt - 1])
            nc.sync.dma_start(out=idx_o[:, : nt - 1], in_=istage[:, : nt - 1])
            nc.scalar.dma_start(out=val_o[:, nt - 1 :], in_=vstage[:, nt - 1 :])
            nc.sync.dma_start(out=idx_o[:, nt - 1 :], in_=istage[:, nt - 1 :])
    if not nc.is_finalized():
        nc.finalize()
    return nc


def _get_nc(t_shard: int) -> "bass.Bass":
    if t_shard not in _NC_CACHE:
        _NC_CACHE[t_shard] = build_nc(t_shard)
    return _NC_CACHE[t_shard]


def _pack_x(x: np.ndarray) -> np.ndarray:
    """[T, K] fp32 -> [P, (T//P) * 6144] u8 (per-block hi fp16 + lo fp8)."""
    t = x.shape[0]
    nt = t // P
    xh = x.astype(np.float16)
    xl = ((x - xh.astype(np.float32)) * np.float32(LO_SCALE)).astype(FP8)
    # x.T is [K, T]; K = c*128 + p, T = b*128 + tok
    hiT = xh.T.reshape(KC, P, nt, P).transpose(1, 2, 0, 3)  # [p, b, c, tok]
    loT = xl.T.reshape(KC, P, nt, P).transpose(1, 2, 0, 3)
    out = np.empty((P, nt, BLK_B), np.uint8)
    out[:, :, :HI_B] = np.ascontiguousarray(hiT).view(np.uint8).reshape(P, nt, HI_B)
    out[:, :, HI_B:] = loT.reshape(P, nt, LO_B).view(np.uint8)
    return out.reshape(P, nt * BLK_B)


def _pack_w(W: np.ndarray) -> np.ndarray:
    """[K, E] fp32 -> [P, 4096] u8: fp16 [c][Wh|Wl'] interleaved."""
    Wh = W.astype(np.float16)
    Wl = ((W - Wh.astype(np.float32)) * np.float32(LO_SCALE)).astype(np.float16)
    # [K, E] -> [KC, P, E] -> [P, KC, E]
    WhP = Wh.reshape(KC, P, E).transpose(1, 0, 2)
    WlP = Wl.reshape(KC, P, E).transpose(1, 0, 2)
    W16 = np.concatenate([WhP, WlP], axis=2)  # [P, KC, 2E] cols 0:64 Wh
    return np.ascontiguousarray(W16).view(np.uint8).reshape(P, W16_B)


def kernel(x: np.ndarray, W_g: np.ndarray, **run_kwargs):
    global LAST_RESULT
    x = np.asarray(x, dtype=np.float32)
    W = np.asarray(W_g, dtype=np.float32)
    t_shard = x.shape[0] // N_CORES
    nc = _get_nc(t_shard)

    Wp = _pack_w(W)
    in_maps = [
        {"xd": _pack_x(x[c * t_shard : (c + 1) * t_shard]), "Wd": Wp}
        for c in range(N_CORES)
    ]
    res = bass_utils.run_bass_kernel_spmd(
        nc, in_maps, core_ids=list(range(N_CORES)), **run_kwargs
    )
    LAST_RESULT = res
    # device layout [P, nt, k]; token t = tile*P + p -> [t_shard, k]
    nt = t_shard // P
    idx = np.concatenate(
        [
            np.moveaxis(r["idx"], 0, 1).reshape(t_shard, 8)[:, :TOPK]
            for r in res.results
        ],
        axis=0,
    ).astype(np.int32)
    val = np.concatenate(
        [np.moveaxis(r["val"], 0, 1).reshape(t_shard, TOPK) for r in res.results],
        axis=0,
    ).astype(np.float32)
    return idx, val


# revision 37
# speedup vs baseline: 1.0184x; 1.0184x over previous
"""MoE gate (router) kernel for Trainium2, 8 NeuronCores, data-parallel.

reference: logits = x @ W_g  ([16384,2048] @ [2048,64]); scores = softmax(logits);
           return top-6 (indices, scores).

Strategy (v2)
-------------
Data-parallel over tokens: each core handles 2048 tokens. The kernel is
HBM-bandwidth bound, so x is shipped at 3 bytes/element instead of 4:

    x  = xh (fp16)  +  2^-11 * xl'          xl' = (x - xh) * 2^11  in fp8 e4m3
    W  = Wh (fp16)  +  2^-11 * Wl'          Wl' = (W - Wh) * 2^11  in fp16
    logits = xh@Wh + 2^-11 * (xh@Wl' + xl'@e4m3(Wh))

All products accumulate exactly in fp32 PSUM; residual logit error ~2^-15
relative (measured: 4/16384 rows with a top-6 order flip, val err ~1e-5).

Per 128-token block (16 blocks/core):
  - ONE dma_start moves a [128 part, 6144 B] u8 slab (4 KiB fp16 hi +
    2 KiB fp8 lo per partition). ALL x slabs stream in order on the sync
    HWDGE ring -- a single ring sustains ~320 GB/s (HBM cap), and keeping
    dma_starts off the scalar engine means ACT work is never head-of-line
    blocked behind a stalled dma instruction. Blocks 12-13 split hi/lo
    into two DMAs and blocks 14-15 into quarters, so the tail matmuls
    chase the stream chunk-by-chunk (last MM retires ~0.7us after the
    final byte instead of ~1.2us). All slabs are SBUF-resident (no
    reuse), so DMA is never back-pressured by compute.
  - 16 matmuls  xh_c @ [Wh|Wl']_c (fp16, N=128) -> PSUM[:, 0:128]
  - 16 matmuls  xl'_c @ Wh8_c     (fp8,  N=64)  -> PSUM[:, 128:192]
  - fold on DVE with a single-AP pair reduce: u = sum over PSUM[:, 64:192]
    viewed as [P, 64, 2]; lg = u * 2^-11 + PSUM[:, :64]  (one PSUM operand
    per DVE op -- PSUM has a single DVE read port).
  - softmax+top6 without max-subtraction (|logits| < ~6): exp+rowsum fused
    on the scalar engine (accum_out) runs IN PARALLEL with DVE
    max8/find_index8 on the raw logits (indices go straight into the
    staging tile); scores = exp(v6) * recip(sum).
  - Outputs staged in SBUF [P, nt, 8]/[P, nt, 6]; two DMAs at the very
    end (mid-stream output DMAs measurably disturb the tile schedule).

Measured: 71.9 us (fp16 hi/lo baseline) -> 52.5-53.5 us (best 52460 ns).
Breakdown: ~8 us fixed framework preamble, ~38.5 us HBM-bound stream @
~327 GB/s, ~4 us compute tail + output DMA latency, ~2 us exit barrier
(the NRT semaphore-zeroing ladder after it is excluded from the reported
exec time).
"""

import os
import sys

import numpy as np

for _p in ("/opt/trn_rl_repo", "/root/.axon_site/_ro/trn_rl_repo"):
    if os.path.isdir(_p) and _p not in sys.path:
        sys.path.insert(0, _p)

import ml_dtypes
import concourse.bass as bass
import concourse.mybir as mybir
from concourse import bacc, bass_utils
from concourse.tile import TileContext

N_CORES = 8
T_FULL = 16384
K = 2048
E = 64
TOPK = 6
P = 128
KC = K // P  # 16 contraction chunks
LO_SCALE = 2048.0  # 2^11
FP8 = ml_dtypes.float8_e4m3

# per-block slab layout (bytes per partition)
HI_B = KC * P * 2  # 4096 B fp16 hi
LO_B = KC * P * 1  # 2048 B fp8 lo
BLK_B = HI_B + LO_B  # 6144
# W slab layout (bytes per partition); the fp8 copy of Wh is derived
# on-device (one DVE cast) instead of spending HBM stream time on it
W16_B = KC * 2 * E * 2  # 4096 B fp16 [Wh|Wl'] interleaved per chunk
W_B = W16_B

_NC_CACHE: dict[int, "bass.Bass"] = {}
LAST_RESULT = None  # BassKernelResults of the most recent kernel() call


def build_nc(t_shard: int = T_FULL // N_CORES) -> "bass.Bass":
    f16 = mybir.dt.float16
    f8 = mybir.dt.float8e4
    f32 = mybir.dt.float32
    u8 = mybir.dt.uint8
    u32 = mybir.dt.uint32
    EXP = mybir.ActivationFunctionType.Exp
    ADD = mybir.AluOpType.add
    MULT = mybir.AluOpType.mult

    assert t_shard % P == 0
    nt = t_shard // P  # number of 128-token blocks

    nc = bacc.Bacc()
    xd = nc.dram_tensor("xd", [P, nt * BLK_B], u8, kind="ExternalInput")
    Wd = nc.dram_tensor("Wd", [P, W_B], u8, kind="ExternalInput")
    idx_o = nc.dram_tensor("idx", [P, nt, 8], u32, kind="ExternalOutput")
    val_o = nc.dram_tensor("val", [P, nt, TOPK], f32, kind="ExternalOutput")

    with TileContext(nc) as tc:
        with (
            tc.tile_pool(name="singles", bufs=1) as singles,
            tc.tile_pool(name="psum", bufs=8, space="PSUM") as psum_pool,
        ):
            # one big resident x slab; 16 DMAs into disjoint slices (deps are
            # range-tracked, so readers only wait on their own block's DMA)
            xall = singles.tile([P, nt * BLK_B], u8)
            W_sb = singles.tile([P, W_B], u8)
            # W alone on the scalar ring (lands fast, scalar engine then only
            # runs ACTs); ALL x blocks stream in order on the sync ring so no
            # compute engine is ever head-of-line blocked by a dma_start.
            nc.scalar.dma_start(out=W_sb, in_=Wd[:])
            for b in range(nt):
                o = b * BLK_B
                if b < nt - 4:
                    nc.sync.dma_start(
                        out=xall[:, o : o + BLK_B], in_=xd[:, o : o + BLK_B]
                    )
                elif b < nt - 2:
                    # tail blocks: hi and lo parts land separately so the
                    # fp16 matmuls can start before the fp8 half arrives
                    nc.sync.dma_start(
                        out=xall[:, o : o + HI_B], in_=xd[:, o : o + HI_B]
                    )
                    nc.sync.dma_start(
                        out=xall[:, o + HI_B : o + BLK_B],
                        in_=xd[:, o + HI_B : o + BLK_B],
                    )
                else:
                    # last 2 blocks: hi in halves, lo in quarters, so the
                    # final matmuls chase the stream piece-by-piece
                    cuts = (
                        0, HI_B // 2, HI_B,
                        HI_B + LO_B // 4, HI_B + LO_B // 2,
                        HI_B + 3 * LO_B // 4, BLK_B,
                    )
                    for q0, q1 in zip(cuts[:-1], cuts[1:]):
                        nc.sync.dma_start(
                            out=xall[:, o + q0 : o + q1],
                            in_=xd[:, o + q0 : o + q1],
                        )
            # [P, KC, 128] fp16: per chunk cols 0:64 = Wh, 64:128 = Wl'
            W16 = W_sb[:, :W16_B].bitcast(f16).rearrange(
                "p (c n) -> p c n", c=KC
            )
            # [P, KC, 64] fp8 = e4m3(Wh), cast on-device (DVE RNE)
            W8_sb = singles.tile([P, KC, E], f8)
            nc.vector.tensor_copy(W8_sb, W16[:, :, :E])
            W8 = W8_sb[:]

            istage = singles.tile([P, nt, 8], u32)
            vstage = singles.tile([P, nt, TOPK], f32)
            # per-block scratch slices of one tile: u(64) lg(64) erow(64)
            # v8(8) ev(8) sume(1) rec(1) -> 212 f32, padded to 224
            SCR = 224
            scratch = singles.tile([P, nt, SCR], f32)

            for b in range(nt):
                xb = xall[:, b * BLK_B : (b + 1) * BLK_B]
                hi = xb[:, :HI_B].bitcast(f16).rearrange("p (c t) -> p c t", c=KC)
                lo = xb[:, HI_B:].bitcast(f8).rearrange("p (c t) -> p c t", c=KC)
                ps = psum_pool.tile([P, 192], f32, tag="ps")
                for c in range(KC):
                    nc.tensor.matmul(
                        ps[:, :128],
                        hi[:, c],
                        W16[:, c],
                        start=(c == 0),
                        stop=(c == KC - 1),
                    )
                for c in range(KC):
                    nc.tensor.matmul(
                        ps[:, 128:],
                        lo[:, c],
                        W8[:, c],
                        start=(c == 0),
                        stop=(c == KC - 1),
                    )
                sc = scratch[:, b]
                u = sc[:, 0:64]
                lg = sc[:, 64:128]
                erow = sc[:, 128:192]
                v8 = sc[:, 192:200]
                ev = sc[:, 200:208]
                sume = sc[:, 208:209]
                rec = sc[:, 209:210]
                # fold: u = ps[:,64:128] + ps[:,128:192] via one strided AP
                nc.vector.tensor_reduce(
                    u,
                    ps[:, 64:].rearrange("p (r j) -> p j r", r=2),
                    axis=mybir.AxisListType.X,
                    op=ADD,
                )
                nc.vector.scalar_tensor_tensor(
                    out=lg,
                    in0=u,
                    scalar=1.0 / LO_SCALE,
                    in1=ps[:, :E],
                    op0=MULT,
                    op1=ADD,
                )
                # softmax + top-6 (no max subtraction; |logits| < ~6).
                # max8/find_index8 run on lg (DVE) IN PARALLEL with the
                # exp+rowsum on the scalar engine; the top-6 VALUES come
                # from a second max8 on erow (exp is monotonic, so the
                # sorted values are exactly exp(v_i) from the same ACT
                # table -- bit-identical to exp'ing v6, one less ACT op
                # and one less cross-engine hop on the val critical path).
                nc.scalar.activation(erow, lg, EXP, accum_out=sume)
                nc.vector.max(out=v8, in_=lg)
                nc.vector.max_index(out=istage[:, b], in_max=v8, in_values=lg)
                nc.vector.max(out=ev, in_=erow)
                nc.vector.reciprocal(rec, sume)
                nc.vector.tensor_scalar_mul(vstage[:, b], ev[:, :TOPK], rec)

            # outputs at the end on the two HWDGE rings (both idle by now).
            # Split bulk (blocks 0..nt-2, deps already satisfied -> descriptor
            # generation overlaps the last block's chain) from a tiny final
            # DMA for block nt-1 that rides the already-warm DGE pipeline.
            nc.scalar.dma_start(out=val_o[:, : nt - 1], in_=vstage[:, : nt - 1])
            nc.sync.dma_start(out=idx_o[:, : nt - 1], in_=istage[:, : nt - 1])
            nc.scalar.dma_start(out=val_o[:, nt - 1 :], in_=vstage[:, nt - 1 :])
            nc.sync.dma_start(out=idx_o[:, nt - 1 :], in_=istage[:, nt - 1 :])
    if not nc.is_finalized():
        nc.finalize()
    return nc


def _get_nc(t_shard: int) -> "bass.Bass":
    if t_shard not in _NC_CACHE:
        _NC_CACHE[t_shard] = build_nc(t_shard)
    return _NC_CACHE[t_shard]


def _pack_x(x: np.ndarray) -> np.ndarray:
    """[T, K] fp32 -> [P, (T//P) * 6144] u8 (per-block hi fp16 + lo fp8)."""
    t = x.shape[0]
    nt = t // P
    xh = x.astype(np.float16)
    xl = ((x - xh.astype(np.float32)) * np.float32(LO_SCALE)).astype(FP8)
    # x.T is [K, T]; K = c*128 + p, T = b*128 + tok
    hiT = xh.T.reshape(KC, P, nt, P).transpose(1, 2, 0, 3)  # [p, b, c, tok]
    loT = xl.T.reshape(KC, P, nt, P).transpose(1, 2, 0, 3)
    out = np.empty((P, nt, BLK_B), np.uint8)
    out[:, :, :HI_B] = np.ascontiguousarray(hiT).view(np.uint8).reshape(P, nt, HI_B)
    out[:, :, HI_B:] = loT.reshape(P, nt, LO_B).view(np.uint8)
    return out.reshape(P, nt * BLK_B)


def _pack_w(W: np.ndarray) -> np.ndarray:
    """[K, E] fp32 -> [P, 4096] u8: fp16 [c][Wh|Wl'] interleaved."""
    Wh = W.astype(np.float16)
    Wl = ((W - Wh.astype(np.float32)) * np.float32(LO_SCALE)).astype(np.float16)
    # [K, E] -> [KC, P, E] -> [P, KC, E]
    WhP = Wh.reshape(KC, P, E).transpose(1, 0, 2)
    WlP = Wl.reshape(KC, P, E).transpose(1, 0, 2)
    W16 = np.concatenate([WhP, WlP], axis=2)  # [P, KC, 2E] cols 0:64 Wh
    return np.ascontiguousarray(W16).view(np.uint8).reshape(P, W16_B)


def kernel(x: np.ndarray, W_g: np.ndarray, **run_kwargs):
    global LAST_RESULT
    x = np.asarray(x, dtype=np.float32)
    W = np.asarray(W_g, dtype=np.float32)
    t_shard = x.shape[0] // N_CORES
    nc = _get_nc(t_shard)

    Wp = _pack_w(W)
    in_maps = [
        {"xd": _pack_x(x[c * t_shard : (c + 1) * t_shard]), "Wd": Wp}
        for c in range(N_CORES)
    ]
    res = bass_utils.run_bass_kernel_spmd(
        nc, in_maps, core_ids=list(range(N_CORES)), **run_kwargs
    )
    LAST_RESULT = res
    # device layout [P, nt, k]; token t = tile*P + p -> [t_shard, k]
    nt = t_shard // P
    idx = np.concatenate(
        [
            np.moveaxis(r["idx"], 0, 1).reshape(t_shard, 8)[:, :TOPK]
            for r in res.results
        ],
        axis=0,
    ).astype(np.int32)
    val = np.concatenate(
        [np.moveaxis(r["val"], 0, 1).reshape(t_shard, TOPK) for r in res.results],
        axis=0,
    ).astype(np.float32)
    return idx, val


# revision 38
# speedup vs baseline: 1.0469x; 1.0281x over previous
"""MoE gate (router) kernel for Trainium2, 8 NeuronCores, data-parallel.

reference: logits = x @ W_g  ([16384,2048] @ [2048,64]); scores = softmax(logits);
           return top-6 (indices, scores).

Strategy (v2)
-------------
Data-parallel over tokens: each core handles 2048 tokens. The kernel is
HBM-bandwidth bound, so x is shipped at 3 bytes/element instead of 4:

    x  = xh (fp16)  +  2^-11 * xl'          xl' = (x - xh) * 2^11  in fp8 e4m3
    W  = Wh (fp16)  +  2^-11 * Wl'          Wl' = (W - Wh) * 2^11  in fp16
    logits = xh@Wh + 2^-11 * (xh@Wl' + xl'@e4m3(Wh))

All products accumulate exactly in fp32 PSUM; residual logit error ~2^-15
relative (measured: 4/16384 rows with a top-6 order flip, val err ~1e-5).

Per 128-token block (16 blocks/core):
  - ONE dma_start moves a [128 part, 6144 B] u8 slab (4 KiB fp16 hi +
    2 KiB fp8 lo per partition). ALL x slabs stream in order on the sync
    HWDGE ring -- a single ring sustains ~320 GB/s (HBM cap), and keeping
    dma_starts off the scalar engine means ACT work is never head-of-line
    blocked behind a stalled dma instruction. Blocks 12-13 split hi/lo
    into two DMAs and blocks 14-15 into quarters, so the tail matmuls
    chase the stream chunk-by-chunk (last MM retires ~0.7us after the
    final byte instead of ~1.2us). All slabs are SBUF-resident (no
    reuse), so DMA is never back-pressured by compute.
  - 16 matmuls  xh_c @ [Wh|Wl']_c (fp16, N=128) -> PSUM[:, 0:128]
  - 16 matmuls  xl'_c @ Wh8_c     (fp8,  N=64)  -> PSUM[:, 128:192]
  - fold on DVE with a single-AP pair reduce: u = sum over PSUM[:, 64:192]
    viewed as [P, 64, 2]; lg = u * 2^-11 + PSUM[:, :64]  (one PSUM operand
    per DVE op -- PSUM has a single DVE read port).
  - softmax+top6 without max-subtraction (|logits| < ~6): exp+rowsum fused
    on the scalar engine (accum_out) runs IN PARALLEL with DVE
    max8/find_index8 on the raw logits (indices go straight into the
    staging tile); scores = exp(v6) * recip(sum).
  - Outputs staged in SBUF [P, nt, 8]/[P, nt, 6]; two DMAs at the very
    end (mid-stream output DMAs measurably disturb the tile schedule).

Measured: 71.9 us (fp16 hi/lo baseline) -> 52.5-53.5 us (best 52460 ns).
Breakdown: ~8 us fixed framework preamble, ~38.5 us HBM-bound stream @
~327 GB/s, ~4 us compute tail + output DMA latency, ~2 us exit barrier
(the NRT semaphore-zeroing ladder after it is excluded from the reported
exec time).
"""

import os
import sys

import numpy as np

for _p in ("/opt/trn_rl_repo", "/root/.axon_site/_ro/trn_rl_repo"):
    if os.path.isdir(_p) and _p not in sys.path:
        sys.path.insert(0, _p)

import ml_dtypes
import concourse.bass as bass
import concourse.mybir as mybir
from concourse import bacc, bass_utils
from concourse.tile import TileContext

N_CORES = 8
T_FULL = 16384
K = 2048
E = 64
TOPK = 6
P = 128
KC = K // P  # 16 contraction chunks
LO_SCALE = 2048.0  # 2^11
FP8 = ml_dtypes.float8_e4m3

# per-block slab layout (bytes per partition)
HI_B = KC * P * 2  # 4096 B fp16 hi
LO_B = KC * P * 1  # 2048 B fp8 lo
BLK_B = HI_B + LO_B  # 6144
# W slab layout (bytes per partition); the fp8 copy of Wh is derived
# on-device (one DVE cast) instead of spending HBM stream time on it
W16_B = KC * 2 * E * 2  # 4096 B fp16 [Wh|Wl'] interleaved per chunk
W_B = W16_B

_NC_CACHE: dict[int, "bass.Bass"] = {}
LAST_RESULT = None  # BassKernelResults of the most recent kernel() call


def build_nc(t_shard: int = T_FULL // N_CORES) -> "bass.Bass":
    f16 = mybir.dt.float16
    f8 = mybir.dt.float8e4
    f32 = mybir.dt.float32
    u8 = mybir.dt.uint8
    u32 = mybir.dt.uint32
    EXP = mybir.ActivationFunctionType.Exp
    ADD = mybir.AluOpType.add
    MULT = mybir.AluOpType.mult

    assert t_shard % P == 0
    nt = t_shard // P  # number of 128-token blocks

    nc = bacc.Bacc()
    xd = nc.dram_tensor("xd", [P, nt * BLK_B], u8, kind="ExternalInput")
    Wd = nc.dram_tensor("Wd", [P, W_B], u8, kind="ExternalInput")
    idx_o = nc.dram_tensor("idx", [P, nt, 8], u32, kind="ExternalOutput")
    val_o = nc.dram_tensor("val", [P, nt, TOPK], f32, kind="ExternalOutput")

    with TileContext(nc) as tc:
        with (
            tc.tile_pool(name="singles", bufs=1) as singles,
            tc.tile_pool(name="psum", bufs=8, space="PSUM") as psum_pool,
        ):
            # one big resident x slab; 16 DMAs into disjoint slices (deps are
            # range-tracked, so readers only wait on their own block's DMA)
            xall = singles.tile([P, nt * BLK_B], u8)
            W_sb = singles.tile([P, W_B], u8)
            # W alone on the scalar ring (lands fast, scalar engine then only
            # runs ACTs); ALL x blocks stream in order on the sync ring so no
            # compute engine is ever head-of-line blocked by a dma_start.
            nc.scalar.dma_start(out=W_sb, in_=Wd[:])
            for b in range(nt):
                o = b * BLK_B
                if b < nt - 4:
                    nc.sync.dma_start(
                        out=xall[:, o : o + BLK_B], in_=xd[:, o : o + BLK_B]
                    )
                else:
                    # tail blocks: hi and lo parts land separately so the
                    # fp16 matmuls can start before the fp8 half arrives.
                    # No finer splits: each extra DMA pays ~0.5us of
                    # serialized desc-gen/completion latency at the ring
                    # tail, which outweighs the overlap gain.
                    nc.sync.dma_start(
                        out=xall[:, o : o + HI_B], in_=xd[:, o : o + HI_B]
                    )
                    nc.sync.dma_start(
                        out=xall[:, o + HI_B : o + BLK_B],
                        in_=xd[:, o + HI_B : o + BLK_B],
                    )
            # [P, KC, 128] fp16: per chunk cols 0:64 = Wh, 64:128 = Wl'
            W16 = W_sb[:, :W16_B].bitcast(f16).rearrange(
                "p (c n) -> p c n", c=KC
            )
            # [P, KC, 64] fp8 = e4m3(Wh), cast on-device (DVE RNE)
            W8_sb = singles.tile([P, KC, E], f8)
            nc.vector.tensor_copy(W8_sb, W16[:, :, :E])
            W8 = W8_sb[:]

            istage = singles.tile([P, nt, 8], u32)
            vstage = singles.tile([P, nt, TOPK], f32)
            # per-block scratch slices of one tile: u(64) lg(64) erow(64)
            # v8(8) ev(8) sume(1) rec(1) -> 212 f32, padded to 224
            SCR = 224
            scratch = singles.tile([P, nt, SCR], f32)

            for b in range(nt):
                xb = xall[:, b * BLK_B : (b + 1) * BLK_B]
                hi = xb[:, :HI_B].bitcast(f16).rearrange("p (c t) -> p c t", c=KC)
                lo = xb[:, HI_B:].bitcast(f8).rearrange("p (c t) -> p c t", c=KC)
                ps = psum_pool.tile([P, 192], f32, tag="ps")
                for c in range(KC):
                    nc.tensor.matmul(
                        ps[:, :128],
                        hi[:, c],
                        W16[:, c],
                        start=(c == 0),
                        stop=(c == KC - 1),
                    )
                for c in range(KC):
                    nc.tensor.matmul(
                        ps[:, 128:],
                        lo[:, c],
                        W8[:, c],
                        start=(c == 0),
                        stop=(c == KC - 1),
                    )
                sc = scratch[:, b]
                u = sc[:, 0:64]
                lg = sc[:, 64:128]
                erow = sc[:, 128:192]
                v8 = sc[:, 192:200]
                ev = sc[:, 200:208]
                sume = sc[:, 208:209]
                rec = sc[:, 209:210]
                # fold: u = ps[:,64:128] + ps[:,128:192] via one strided AP
                nc.vector.tensor_reduce(
                    u,
                    ps[:, 64:].rearrange("p (r j) -> p j r", r=2),
                    axis=mybir.AxisListType.X,
                    op=ADD,
                )
                nc.vector.scalar_tensor_tensor(
                    out=lg,
                    in0=u,
                    scalar=1.0 / LO_SCALE,
                    in1=ps[:, :E],
                    op0=MULT,
                    op1=ADD,
                )
                # softmax + top-6 (no max subtraction; |logits| < ~6).
                # max8/find_index8 run on lg (DVE) IN PARALLEL with the
                # exp+rowsum on the scalar engine; the top-6 VALUES come
                # from a second max8 on erow (exp is monotonic, so the
                # sorted values are exactly exp(v_i) from the same ACT
                # table -- bit-identical to exp'ing v6, one less ACT op
                # and one less cross-engine hop on the val critical path).
                nc.scalar.activation(erow, lg, EXP, accum_out=sume)
                nc.vector.max(out=v8, in_=lg)
                nc.vector.max_index(out=istage[:, b], in_max=v8, in_values=lg)
                nc.vector.max(out=ev, in_=erow)
                nc.vector.reciprocal(rec, sume)
                nc.vector.tensor_scalar_mul(vstage[:, b], ev[:, :TOPK], rec)

            # outputs at the end on the two HWDGE rings (both idle by now).
            # Split bulk (blocks 0..nt-2, deps already satisfied -> descriptor
            # generation overlaps the last block's chain) from a tiny final
            # DMA for block nt-1 that rides the already-warm DGE pipeline.
            nc.scalar.dma_start(out=val_o[:, : nt - 1], in_=vstage[:, : nt - 1])
            nc.sync.dma_start(out=idx_o[:, : nt - 1], in_=istage[:, : nt - 1])
            nc.scalar.dma_start(out=val_o[:, nt - 1 :], in_=vstage[:, nt - 1 :])
            nc.sync.dma_start(out=idx_o[:, nt - 1 :], in_=istage[:, nt - 1 :])
    if not nc.is_finalized():
        nc.finalize()
    return nc


def _get_nc(t_shard: int) -> "bass.Bass":
    if t_shard not in _NC_CACHE:
        _NC_CACHE[t_shard] = build_nc(t_shard)
    return _NC_CACHE[t_shard]


def _pack_x(x: np.ndarray) -> np.ndarray:
    """[T, K] fp32 -> [P, (T//P) * 6144] u8 (per-block hi fp16 + lo fp8)."""
    t = x.shape[0]
    nt = t // P
    xh = x.astype(np.float16)
    xl = ((x - xh.astype(np.float32)) * np.float32(LO_SCALE)).astype(FP8)
    # x.T is [K, T]; K = c*128 + p, T = b*128 + tok
    hiT = xh.T.reshape(KC, P, nt, P).transpose(1, 2, 0, 3)  # [p, b, c, tok]
    loT = xl.T.reshape(KC, P, nt, P).transpose(1, 2, 0, 3)
    out = np.empty((P, nt, BLK_B), np.uint8)
    out[:, :, :HI_B] = np.ascontiguousarray(hiT).view(np.uint8).reshape(P, nt, HI_B)
    out[:, :, HI_B:] = loT.reshape(P, nt, LO_B).view(np.uint8)
    return out.reshape(P, nt * BLK_B)


def _pack_w(W: np.ndarray) -> np.ndarray:
    """[K, E] fp32 -> [P, 4096] u8: fp16 [c][Wh|Wl'] interleaved."""
    Wh = W.astype(np.float16)
    Wl = ((W - Wh.astype(np.float32)) * np.float32(LO_SCALE)).astype(np.float16)
    # [K, E] -> [KC, P, E] -> [P, KC, E]
    WhP = Wh.reshape(KC, P, E).transpose(1, 0, 2)
    WlP = Wl.reshape(KC, P, E).transpose(1, 0, 2)
    W16 = np.concatenate([WhP, WlP], axis=2)  # [P, KC, 2E] cols 0:64 Wh
    return np.ascontiguousarray(W16).view(np.uint8).reshape(P, W16_B)


def kernel(x: np.ndarray, W_g: np.ndarray, **run_kwargs):
    global LAST_RESULT
    x = np.asarray(x, dtype=np.float32)
    W = np.asarray(W_g, dtype=np.float32)
    t_shard = x.shape[0] // N_CORES
    nc = _get_nc(t_shard)

    Wp = _pack_w(W)
    in_maps = [
        {"xd": _pack_x(x[c * t_shard : (c + 1) * t_shard]), "Wd": Wp}
        for c in range(N_CORES)
    ]
    res = bass_utils.run_bass_kernel_spmd(
        nc, in_maps, core_ids=list(range(N_CORES)), **run_kwargs
    )
    LAST_RESULT = res
    # device layout [P, nt, k]; token t = tile*P + p -> [t_shard, k]
    nt = t_shard // P
    idx = np.concatenate(
        [
            np.moveaxis(r["idx"], 0, 1).reshape(t_shard, 8)[:, :TOPK]
            for r in res.results
        ],
        axis=0,
    ).astype(np.int32)
    val = np.concatenate(
        [np.moveaxis(r["val"], 0, 1).reshape(t_shard, TOPK) for r in res.results],
        axis=0,
    ).astype(np.float32)
    return idx, val
